# revision 54
# baseline (speedup 1.0000x reference)
"""Trainium2 Bass kernel for nn_RankingSet (retrieval_knn, cosine threshold count).

Computes, for each query q:
    ct[q] = #{ m : cos_sim(data[m], qn[q]) >= thresh[q] - tol[q] } - 1
where thresh[q] = <qn[q], tn[q]> (normalized query/truth dot), and
tol = ATOL + RTOL*|thresh| (torch.isclose semantics folded into a single
one-sided comparison: (s >= t) | (|s-t| <= tol)  ==  s >= t - tol).

Strategy (8 NeuronCores, SPMD), v4 "fp8 host-packed, fine-grained drain":
  - data (500000, 512) f32 sharded row-wise; each core gets 62500 rows =
    30 blocks x 2048 rows + 1 tail block x 1060 (no pad).
  - Host casts data to fp8e4m3 (unscaled) and packs each core's shard
    block-major into the matmul-ready flat layout
        A[(i, p, j, m)] = fp8(data[m0_i + m, 128j + p])
    so the device does NO transposes, reads 1 byte/elem from HBM (4x
    less than f32), and every per-block DMA is one linear HBM region
    with 4-8KB contiguous runs per partition. A single SP HWDGE queue
    saturates the measured per-core DMA bandwidth (~330 GB/s).
  - Queries are L2-normalized on host, scaled by 16, cast to fp8, and
    shipped pre-transposed as qT[p, j, q] = fp8(16*qn[q, 128j + p]).
    Threshold tau = 16*(thresh - tol) matches the scaling.
  - Per block on device: one DMA of the [128, 4, w] fp8 tile, then per
    HALF_W(=512)-wide piece: 2 fp8 DoubleRow matmuls (each contracts
    K=256) into a single-bank PSUM tile, drained immediately by a
    compare+count op alternating between the DVE (tensor_scalar is_ge,
    accum_out) and the ACT engine (Sign activation with bias=-tau,
    accumulator: sum sign(sim-tau) = 2*count - w). Single-bank PSUM
    granularity (8 tiles in flight) keeps compare latency off the
    matmul critical path.
  - Host sums the per-piece count columns, fixes up the sign-sum
    columns, and subtracts the self row.
"""

import sys

import numpy as np

for _p in ("/opt/trn_rl_repo",):
    if _p not in sys.path:
        sys.path.insert(0, _p)

N_TOTAL = 500000
D = 512
Q = 128
N_CORES = 8
ROWS_PER_CORE = N_TOTAL // N_CORES  # 62500

W_FULL = 2048
N_FULL = 30
W_TAIL = ROWS_PER_CORE - N_FULL * W_FULL  # 1060: tail is exact, no pad rows
BLK_WIDTHS = [W_FULL] * N_FULL + [W_TAIL]
N_BLOCKS = len(BLK_WIDTHS)  # 31
M_PAD = sum(BLK_WIDTHS)  # 62500
N_PAD = M_PAD - ROWS_PER_CORE  # 0
L_FLAT = 4 * M_PAD  # fp8 bytes per partition: 250000
BLK_OFFS = [4 * sum(BLK_WIDTHS[:i]) for i in range(N_BLOCKS)]

RTOL = 1e-5
ATOL = 1e-8

# Compare granularity: each block's PSUM is drained in HALF_W-wide pieces so
# the compare latency pipelines against the next matmuls (whole-block
# compares put ~8 us of PSUM-release serialization on the critical path;
# 512 = single-bank PSUM tiles, 8 in flight, best measured pipelining).
HALF_W = 512


def cnt_columns(half_w=HALF_W):
    """(block, offset, width) per count column, in emission order."""
    cols = []
    for i, w in enumerate(BLK_WIDTHS):
        for h0 in range(0, w, half_w):
            cols.append((i, h0, min(half_w, w - h0)))
    return cols

# Data is NOT pre-scaled (fp8 subnormals below 2^-6 quantize small values
# about as well as the normal-range relative rounding there, and skipping
# the 1 GB multiply saves ~0.7 s of single-CPU host time). Queries are
# scaled by 16 so the unit-norm entries (~0.044) stay in fp8 normal range.
S_DATA = 1.0
S_Q = 16.0
S_SIM = S_DATA * S_Q  # 16


def _fp8():
    import ml_dtypes

    return ml_dtypes.float8_e4m3


def host_tau(queries, truths):
    """Per-query scaled threshold tau = (thresh - tol) * S_SIM, and qn (f64)."""
    q = queries.astype(np.float64)
    t = truths.astype(np.float64)
    nq = np.maximum(np.linalg.norm(q, axis=1), 1e-12)
    nt = np.maximum(np.linalg.norm(t, axis=1), 1e-12)
    thresh = np.sum(q * t, axis=1) / (nq * nt)
    tol = ATOL + RTOL * np.abs(thresh)
    tau = ((thresh - tol) * S_SIM).astype(np.float32)
    qn = q / nq[:, None]
    return tau, qn


def host_pack_queries(qn):
    """qT[p, j, q] = fp8(S_Q * qn[q, 128j + p]) as a [128, 4, Q] array."""
    fp8 = _fp8()
    qT = (qn.T * S_Q).astype(np.float32).astype(fp8)  # [512, Q]
    return np.ascontiguousarray(qT.reshape(4, 128, Q).transpose(1, 0, 2))


def host_pack_data(data, layout="bmaj"):
    """Per-core packed fp8 banks.

    layout='pmaj': [128, L_FLAT] with
        A[p, BLK_OFFS[i] + j*w_i + m] = fp8(S_DATA * data[c0 + m0_i + m, 128j + p])
    layout='bmaj': flat [128 * L_FLAT] block-major so every per-block DMA
        is one linear HBM region:
        A[128*BLK_OFFS[i] + p*4*w_i + j*w_i + m] = same element.

    Returns a list of 8 arrays (zero-padded rows). Scale+cast+pack run in
    parallel threads."""
    from concurrent.futures import ThreadPoolExecutor

    fp8 = _fp8()
    if layout == "pmaj":
        packs = [np.empty((128, L_FLAT), dtype=fp8) for _ in range(N_CORES)]
    else:
        packs = [np.empty(128 * L_FLAT, dtype=fp8) for _ in range(N_CORES)]
    full_rows = N_FULL * W_FULL  # 61440
    n_sub = 5  # full blocks per task: 30 = 5 tasks x 6 blocks

    MB = 64  # m-chunk: cast output stays cache-hot for the permute

    def fill(c, i0, nb, w, src):
        # src: [nb*w, 512] f32 (zero-padded) for blocks i0..i0+nb-1;
        # fused chunked cast+permute.
        if layout == "pmaj":
            dst = packs[c][:, BLK_OFFS[i0] : BLK_OFFS[i0] + 4 * nb * w].reshape(
                128, nb, 4, w
            )
        else:
            dst = packs[c][
                128 * BLK_OFFS[i0] : 128 * (BLK_OFFS[i0] + 4 * nb * w)
            ].reshape(nb, 128, 4, w)
        for b in range(nb):
            db = dst[:, b] if layout == "pmaj" else dst[b]
            for m0 in range(0, w, MB):
                chunk = src[b * w + m0 : b * w + m0 + MB]
                if S_DATA != 1.0:
                    chunk = np.multiply(chunk, S_DATA)
                c8 = chunk.astype(fp8)
                db[:, :, m0 : m0 + MB] = c8.reshape(-1, 4, 128).transpose(2, 1, 0)

    def work_full(task):
        c, s = divmod(task, n_sub)
        nb = N_FULL // n_sub  # 6 blocks
        r0 = s * nb * W_FULL
        rows = nb * W_FULL
        shard = data[c * ROWS_PER_CORE + r0 : c * ROWS_PER_CORE + r0 + rows]
        fill(c, s * nb, nb, W_FULL, shard)

    def work_tail(c):
        shard = data[c * ROWS_PER_CORE + full_rows : (c + 1) * ROWS_PER_CORE]
        fill(c, N_FULL, 1, W_TAIL, shard)

    with ThreadPoolExecutor(16) as ex:
        futs = [ex.submit(work_full, t) for t in range(N_CORES * n_sub)]
        futs += [ex.submit(work_tail, c) for c in range(N_CORES)]
        for f in futs:
            f.result()
    return packs


def host_pack_core(data, c):
    """Pack a single core's shard (bmaj flat [128*L_FLAT] fp8)."""
    fp8 = _fp8()
    pack = np.empty(128 * L_FLAT, dtype=fp8)
    MB = 64
    shard = data[c * ROWS_PER_CORE : (c + 1) * ROWS_PER_CORE]
    for i, w in enumerate(BLK_WIDTHS):
        dst = pack[128 * BLK_OFFS[i] : 128 * (BLK_OFFS[i] + 4 * w)].reshape(
            128, 4, w
        )
        r0 = i * W_FULL
        for m0 in range(0, w, MB):
            c8 = shard[r0 + m0 : r0 + min(m0 + MB, w)].astype(fp8)
            dst[:, :, m0 : m0 + MB] = c8.reshape(-1, 4, 128).transpose(2, 1, 0)
    return pack


def build_nc2(
    repeat=1,
    debug=False,
    cmp_engines=("vector", "scalar"),
    hw_loop=False,
    dma_split=False,
    only=None,
    layout="bmaj",
    chunk_bufs=4,
    half_w=HALF_W,
    mask_fp8=True,
    wpair=1,
    mask_psum=False,
):
    """Build + compile the per-core Bass program (v3 fp8 flat).

    repeat > 1 re-runs the whole scan that many times over the same data
    (for amortized wall-clock timing; results identical). hw_loop uses a
    For_i hardware loop instead of python unrolling. dma_split: False =
    one HWDGE queue (SP); True = each block's two j-pair DMAs on SP +
    ACT queues; 3 = whole-block DMAs round-robin over SP/ACT/Pool
    queues. only: 'dma' or 'pe' builds an isolation variant for
    roofline measurement (results are garbage)."""
    import concourse.bacc as bacc
    from concourse import mybir, tile
    from contextlib import ExitStack

    f32 = mybir.dt.float32
    bf16 = mybir.dt.bfloat16
    fp8 = mybir.dt.float8e4
    Alu = mybir.AluOpType
    Act = mybir.ActivationFunctionType
    DR = mybir.MatmulPerfMode.DoubleRow

    nc = bacc.Bacc("TRN2", target_bir_lowering=False, debug=debug)

    if layout == "bmaj":
        data_d = nc.dram_tensor(
            "data", [128 * L_FLAT], fp8, kind="ExternalInput"
        ).ap()
    else:
        data_d = nc.dram_tensor(
            "data", [128, L_FLAT], fp8, kind="ExternalInput"
        ).ap()
    q_d = nc.dram_tensor("qT", [128, 4, Q], fp8, kind="ExternalInput").ap()
    # col 0: +tau (DVE is_ge operand), col 1: -tau (ACT Sign bias)
    tau_d = nc.dram_tensor("tau", [Q, 2], f32, kind="ExternalInput").ap()
    cols = cnt_columns(half_w)
    n_cols = len(cols)
    out_d = nc.dram_tensor("cnt", [Q, n_cols], f32, kind="ExternalOutput").ap()

    with ExitStack() as ctx:
        tc = ctx.enter_context(tile.TileContext(nc))
        const = ctx.enter_context(tc.tile_pool(name="const", bufs=1))
        chunks = ctx.enter_context(tc.tile_pool(name="chunks", bufs=chunk_bufs))
        psum_bufs = max(2, 8 // -(-(half_w * 4) // 2048))  # use all 8 banks
        if mask_psum:
            psum_bufs = max(2, psum_bufs - 2)  # leave 2 banks for masks
        psum = ctx.enter_context(
            tc.tile_pool(name="psum", bufs=psum_bufs, space="PSUM")
        )
        pscr = (
            ctx.enter_context(tc.tile_pool(name="pscr", bufs=2, space="PSUM"))
            if mask_psum
            else None
        )
        scratch = ctx.enter_context(tc.tile_pool(name="scratch", bufs=2))

        qT = const.tile([128, 4, Q], fp8)
        nc.sync.dma_start(qT[:], q_d[:])
        taus = const.tile([Q, 2], f32)
        nc.sync.dma_start(taus[:], tau_d[:])
        cnt = const.tile([Q, n_cols], f32)
        if only:
            nc.vector.memset(cnt[:], 0.0)
        tconst = None
        if only == "pe":
            tconst = const.tile([128, 4, W_FULL], fp8)
            nc.vector.memset(tconst[:], 0.0)

        dma_engines = {0: nc.sync, 1: nc.scalar, 2: nc.gpsimd}

        def body():
            col = 0
            for i in range(N_BLOCKS):
                w = BLK_WIDTHS[i]
                off = BLK_OFFS[i]
                if only == "pe":
                    w = W_FULL
                    t = tconst
                    ps = psum.tile([128, w], f32, tag="ps")
                    for h in range(w // 512):
                        sl = slice(h * 512, (h + 1) * 512)
                        nc.tensor.matmul(
                            ps[:, sl], qT[:, 0:2, :], t[:, 0:2, sl],
                            start=True, stop=False, perf_mode=DR,
                        )
                    for h in range(w // 512):
                        sl = slice(h * 512, (h + 1) * 512)
                        nc.tensor.matmul(
                            ps[:, sl], qT[:, 2:4, :], t[:, 2:4, sl],
                            start=False, stop=True, perf_mode=DR,
                        )
                    tiny = scratch.tile([128, 16], f32, tag="tiny")
                    nc.vector.tensor_copy(tiny[:], ps[:, 0:16])
                    continue
                t = chunks.tile([128, 4, w], fp8, tag="blk")
                if layout == "bmaj":
                    blk_src = data_d[128 * off : 128 * (off + 4 * w)].rearrange(
                        "(p j m) -> p j m", p=128, j=4
                    )
                else:
                    blk_src = data_d[:, off : off + 4 * w].rearrange(
                        "p (j m) -> p j m", j=4
                    )
                if dma_split == 3:
                    dma_engines[i % 3].dma_start(t[:], blk_src)
                elif dma_split:
                    nc.sync.dma_start(t[:, 0:2, :], blk_src[:, 0:2, :])
                    nc.scalar.dma_start(t[:, 2:4, :], blk_src[:, 2:4, :])
                else:
                    nc.sync.dma_start(t[:], blk_src)
                if only == "dma":
                    tiny = scratch.tile([128, 16], fp8, tag="tinyd")
                    nc.vector.tensor_copy(tiny[:], t[:, 0, 0:16])
                    continue
                # Per HALF_W-wide piece: A-pass (d 0..255) then B-pass
                # (d 256..511) into a single-bank PSUM tile, drained
                # immediately by a compare on the alternating engine.
                # Pieces are processed in groups of `wpair` sharing each
                # PE weight load (A-matmuls for the whole group, then
                # B-matmuls, then drains).
                mdt = f32 if mask_psum else (fp8 if mask_fp8 else bf16)
                pieces = [
                    (h0, min(half_w, w - h0)) for h0 in range(0, w, half_w)
                ]
                for g0 in range(0, len(pieces), wpair):
                    group = pieces[g0 : g0 + wpair]
                    phs = []
                    for h0, wh in group:
                        ph = psum.tile([128, wh], f32, tag="ps")
                        phs.append(ph)
                        for a in range(0, wh, 512):
                            sr = slice(a, min(a + 512, wh))
                            sa = slice(h0 + a, h0 + min(a + 512, wh))
                            nc.tensor.matmul(
                                ph[:, sr], qT[:, 0:2, :], t[:, 0:2, sa],
                                start=True, stop=False, perf_mode=DR,
                            )
                    for ph, (h0, wh) in zip(phs, group):
                        for a in range(0, wh, 512):
                            sr = slice(a, min(a + 512, wh))
                            sa = slice(h0 + a, h0 + min(a + 512, wh))
                            nc.tensor.matmul(
                                ph[:, sr], qT[:, 2:4, :], t[:, 2:4, sa],
                                start=False, stop=True, perf_mode=DR,
                            )
                    for ph, (h0, wh) in zip(phs, group):
                        if only == "nocmp":
                            tiny = scratch.tile([128, 16], f32, tag="tinyn")
                            nc.vector.tensor_copy(tiny[:], ph[:, 0:16])
                            col += 1
                            continue
                        eng = cmp_engines[col % len(cmp_engines)]
                        mpool = pscr if mask_psum else scratch
                        mtag = "mask" if mask_psum else ("maskV", "maskA")
                        if eng == "vector":
                            mask = mpool.tile(
                                [128, wh], mdt,
                                tag=mtag if mask_psum else mtag[0],
                            )
                            nc.vector.tensor_scalar(
                                mask[:], ph[:], taus[:, 0:1], None,
                                op0=Alu.is_ge, op1=Alu.add,
                                accum_out=cnt[:, col : col + 1],
                            )
                        else:
                            sgn = mpool.tile(
                                [128, wh], mdt,
                                tag=mtag if mask_psum else mtag[1],
                            )
                            nc.scalar.activation(
                                sgn[:], ph[:], Act.Sign,
                                bias=taus[:, 1:2], scale=1.0,
                                accum_out=cnt[:, col : col + 1],
                            )
                        col += 1

        if hw_loop and repeat > 1:
            with tc.For_i(0, repeat):
                body()
        else:
            for r in range(repeat):
                body()

        nc.sync.dma_start(out_d[:], cnt[:])

    nc.compile()
    return nc


_CACHED_NC = None
_CACHED_ENGINES = ("vector", "scalar")
_LAST_EXEC_NS = None


def counts_from_raw(cnt_raw, tau, half_w=HALF_W):
    """Host fixup: cnt_raw [n_cores, Q, n_cols] f32 -> int32 counts [Q]."""
    cmp_engines = _CACHED_ENGINES
    total = np.zeros(Q, dtype=np.float64)
    for k, (_i, _h0, wh) in enumerate(cnt_columns(half_w)):
        col = cnt_raw[:, :, k].sum(axis=0)  # over cores
        if cmp_engines[k % len(cmp_engines)] == "vector":
            total += col
        else:
            # col = sum sign(sim - tau) = #above - #below over wh rows/core
            total += (col + N_CORES * wh) / 2.0
    return np.round(total - 1.0).astype(np.int32)


_RUNNER = None


def _get_runner(nc):
    """Build (once) a non-donating jitted SPMD runner for nc."""
    global _RUNNER
    if _RUNNER is not None:
        return _RUNNER
    import jax
    from jax.sharding import Mesh, PartitionSpec, NamedSharding
    from jax.experimental.shard_map import shard_map
    from concourse import mybir
    from concourse.bass2jax import (
        _bass_exec_p,
        install_neuronx_cc_hook,
        partition_id_tensor,
    )

    install_neuronx_cc_hook()
    partition_name = (
        nc.partition_id_tensor.name if nc.partition_id_tensor else None
    )
    in_names, out_names, out_avals, zero_outs = [], [], [], []
    for alloc in nc.m.functions[0].allocations:
        if not isinstance(alloc, mybir.MemoryLocationSet):
            continue
        name = alloc.memorylocations[0].name
        if alloc.kind == "ExternalInput":
            if name != partition_name:
                in_names.append(name)
        elif alloc.kind == "ExternalOutput":
            out_names.append(name)
            shape = tuple(alloc.tensor_shape)
            dtype = mybir.dt.np(alloc.dtype)
            out_avals.append(jax.core.ShapedArray(shape, dtype))
            zero_outs.append(np.zeros(shape, dtype))
    all_names = in_names + out_names
    if partition_name is not None:
        all_names = all_names + [partition_name]

    def _body(*args):
        operands = list(args)
        if partition_name is not None:
            operands.append(partition_id_tensor())
        return tuple(
            _bass_exec_p.bind(
                *operands,
                out_avals=tuple(out_avals),
                in_names=tuple(all_names),
                out_names=tuple(out_names),
                lowering_input_output_aliases=(),
                sim_require_finite=True,
                sim_require_nnan=True,
                nc=nc,
            )
        )

    devices = jax.devices()[:N_CORES]
    mesh = Mesh(np.asarray(devices), ("core",))
    spec = PartitionSpec("core")
    n_args = len(in_names) + len(out_names)
    fn = jax.jit(
        shard_map(
            _body, mesh=mesh, in_specs=(spec,) * n_args,
            out_specs=(spec,) * len(out_names), check_rep=False,
        ),
        keep_unused=True,
    )
    sh = NamedSharding(mesh, spec)
    _RUNNER = (fn, devices, sh, in_names, out_names, out_avals, zero_outs)
    return _RUNNER


def kernel(data, queries, truths):
    global _CACHED_NC, _LAST_EXEC_NS

    data = np.ascontiguousarray(data, dtype=np.float32)
    queries = np.ascontiguousarray(queries, dtype=np.float32)
    truths = np.ascontiguousarray(truths, dtype=np.float32)

    if _CACHED_NC is None:
        _CACHED_NC = build_nc2(cmp_engines=_CACHED_ENGINES)
    nc = _CACHED_NC

    tau, qn = host_tau(queries, truths)
    qT8 = host_pack_queries(qn)
    tau2 = np.stack([tau, -tau], axis=1).astype(np.float32)  # [Q, 2]

    try:
        import jax

        fn, devices, sh, in_names, out_names, out_avals, zero_outs = (
            _get_runner(nc)
        )
        # Pack core-by-core, strictly serially: device_put is async, so the
        # tunnel transfer of core c overlaps the CPU packing of core c+1.
        # (A thread pool here is a trap: fair-scheduled concurrent packs mean
        # no shard finishes early and the transfer overlap disappears.)
        shards = [
            jax.device_put(host_pack_core(data, c), devices[c])
            for c in range(N_CORES)
        ]
        data_g = jax.make_array_from_single_device_arrays(
            (N_CORES * 128 * L_FLAT,), sh, shards
        )
        small = {
            "qT": np.concatenate([qT8] * N_CORES, axis=0),
            "tau": np.concatenate([tau2] * N_CORES, axis=0),
        }
        args = []
        for name in in_names:
            args.append(data_g if name == "data" else jax.device_put(small[name], sh))
        for z in zero_outs:
            args.append(
                jax.device_put(
                    np.zeros((N_CORES * z.shape[0], *z.shape[1:]), z.dtype), sh
                )
            )
        out = fn(*args)
        cnt_raw = np.asarray(out[0]).reshape(
            N_CORES, *out_avals[0].shape
        )
    except Exception:
        # Fallback: the generic SPMD path.
        from concourse import bass_utils

        packs = host_pack_data(data)
        in_maps = [
            {"data": packs[c], "qT": qT8, "tau": tau2} for c in range(N_CORES)
        ]
        res = bass_utils.run_bass_kernel_spmd(
            nc, in_maps, core_ids=list(range(N_CORES))
        )
        _LAST_EXEC_NS = res.exec_time_ns
        cnt_raw = np.stack([r["cnt"] for r in res.results], axis=0)
    return counts_from_raw(cnt_raw, tau)
